# revision 6
# baseline (speedup 1.0000x reference)
"""Trainium2 Bass kernel for nn_Net_18262200943034 (stereo cost-volume soft-argmin).

Math (validated vs reference at 7e-7 rel err):
  vol[b,d,h,w] = [w>=d] * (SL[b,h,w] + SR[b,h,w-d]),  SL/SR = channel-means
  out = soft-argmin over d' of trilinear-x4-upsampled vol  -> [B, 4H, 4W]

Per core (8 cores = batch 2 x four 64-row h' blocks):
  1. fused C-mean + H-interp as matmuls -> SLH^T [128w, 64h'], SRH [64h', 128w]
  2. Toeplitz DMA from zero-padded SRH in DRAM -> masked shifted term for all (h',d)
  3. W-upsample and D-upsample as matmuls (interp matrices as inputs)
  4. exp on ACT (values bounded, no max-subtract needed), softmax + soft-argmin
     reductions on DVE in big batched ops.

All per-core inputs travel in ONE [128, 2736] "mega" tensor so a single DMA
semaphore covers every constant (matmuls tolerate only one sync wait).
"""
import os
import numpy as np

import concourse.bacc as bacc
import concourse.bass as bass
import concourse.mybir as mybir
import concourse.tile as tile
from concourse.bass_utils import run_bass_kernel_spmd

F32 = mybir.dt.float32

B, C, H, W = 2, 32, 64, 128
D, DP = 48, 192
H4, W4 = 256, 512
HB = 64            # h' rows per core
HS = 18            # source h rows needed
HPAD = 20          # padded so C*HPAD = 640 = 5*128
KCH = 5            # K chunks of 128 for the (c,h) contraction
H_START = [0, 15, 31, 47]

# mega layout (free-dim offsets, fp32 elements)
OFF_LP, OFF_AT, OFF_RP = 0, 640, 960
OFF_MASK, OFF_V, OFF_UBLK, OFF_DVEC = 1600, 1648, 2160, 2544
MEGA_F = 2736


def _interp_pairs(in_size, out_size):
    src = (np.arange(out_size, dtype=np.float32)
           * np.float32((in_size - 1) / (out_size - 1)))
    i0 = np.clip(np.floor(src).astype(np.int32), 0, in_size - 1)
    i1 = np.clip(i0 + 1, 0, in_size - 1)
    w = (src - i0.astype(np.float32)).astype(np.float32)
    return i0, i1, w


def _interp_matrix(in_size, out_size):
    i0, i1, w = _interp_pairs(in_size, out_size)
    M = np.zeros((in_size, out_size), dtype=np.float32)
    for o in range(out_size):
        M[i0[o], o] += np.float32(1.0) - w[o]
        M[i1[o], o] += w[o]
    return M


def _shared_mega():
    """The core-independent part of the mega input."""
    mega = np.zeros((128, MEGA_F), np.float32)
    mega[:, OFF_MASK:OFF_MASK + D] = (
        np.arange(W)[:, None] >= (D - 1 - np.arange(D))[None, :]).astype(np.float32)
    mega[:, OFF_V:OFF_V + W4] = _interp_matrix(W, W4)
    U_rev = _interp_matrix(D, DP)[::-1]
    mega[0:48, OFF_UBLK:OFF_UBLK + DP] = U_rev
    mega[48:96, OFF_UBLK + DP:OFF_UBLK + 2 * DP] = U_rev
    mega[:, OFF_DVEC:OFF_DVEC + DP] = np.arange(DP, dtype=np.float32)[None, :]
    return mega


def _core_mega(shared, left, right, b, j):
    hs = H_START[j]
    nvalid = min(H, hs + HS) - hs
    lp = np.zeros((C, HPAD, W), np.float32)
    rp = np.zeros((C, HPAD, W), np.float32)
    lp[:, :nvalid] = left[b, :, hs:hs + nvalid]
    rp[:, :nvalid] = right[b, :, hs:hs + nvalid]

    i0, i1, w = _interp_pairs(H, H4)
    A = np.zeros((HB, HPAD), np.float32)
    inv2c = np.float32(1.0 / (2 * C))
    for i in range(HB):
        hp = HB * j + i
        A[i, i0[hp] - hs] += (np.float32(1.0) - w[hp]) * inv2c
        A[i, i1[hp] - hs] += w[hp] * inv2c
    aT = np.ascontiguousarray(
        np.broadcast_to(A.T[None], (C, HPAD, HB))).reshape(KCH, 128, HB)

    mega = shared.copy()
    # [(k p), x] -> [p, (k x)]
    mega[:, OFF_LP:OFF_LP + 640] = (
        lp.reshape(KCH, 128, W).transpose(1, 0, 2).reshape(128, KCH * W))
    mega[:, OFF_RP:OFF_RP + 640] = (
        rp.reshape(KCH, 128, W).transpose(1, 0, 2).reshape(128, KCH * W))
    mega[:, OFF_AT:OFF_AT + 320] = (
        aT.transpose(1, 0, 2).reshape(128, KCH * HB))
    return mega


def build_nc():
    nc = bacc.Bacc("TRN2", target_bir_lowering=False, debug=False)

    mega_d = nc.declare_dram_parameter("mega", [128, MEGA_F], F32, isOutput=False)
    outt_d = nc.declare_dram_parameter("outt", [HB, W4], F32, isOutput=True)
    srhp_dram = nc.dram_tensor("srhp", [HB, D + W], F32)  # zero-padded SRH

    EXP = mybir.ActivationFunctionType.Exp
    AX = mybir.AxisListType.X

    with tile.TileContext(nc) as tc:
        with (
            tc.tile_pool(name="consts", bufs=1) as cpool,
            tc.tile_pool(name="psA", bufs=1, space="PSUM") as psA,
        ):
            mega_sb = cpool.tile([128, MEGA_F], F32)
            nc.sync.dma_start(mega_sb[:], mega_d[:])

            lp_v = mega_sb[:, OFF_LP:OFF_LP + 640].rearrange("p (k w) -> p k w", k=KCH)
            rp_v = mega_sb[:, OFF_RP:OFF_RP + 640].rearrange("p (k w) -> p k w", k=KCH)
            aT_v = mega_sb[:, OFF_AT:OFF_AT + 320].rearrange("p (k m) -> p k m", k=KCH)
            mask_v = mega_sb[:, OFF_MASK:OFF_MASK + D]
            v_v = mega_sb[:, OFF_V:OFF_V + W4]
            ublk_v = mega_sb[0:96, OFF_UBLK:OFF_UBLK + 2 * DP]
            dvec_v = mega_sb[:, OFF_DVEC:OFF_DVEC + DP]

            # Stage A: SLH^T = lp^T @ aT  (contract (c,h)),  SRH = aT^T @ rp
            slht_ps = psA.tile([W, HB], F32)
            srh_ps = psA.tile([HB, W], F32)
            for k in range(KCH):
                nc.tensor.matmul(slht_ps[:], lp_v[:, k, :], aT_v[:, k, :],
                                 start=(k == 0), stop=(k == KCH - 1))
            for k in range(KCH):
                nc.tensor.matmul(srh_ps[:], aT_v[:, k, :], rp_v[:, k, :],
                                 start=(k == 0), stop=(k == KCH - 1))

            slht_sb = cpool.tile([W, HB], F32)
            nc.vector.tensor_copy(slht_sb[:], slht_ps[:])
            srhp_sb = cpool.tile([HB, D + W], F32)
            nc.vector.memset(srhp_sb[:, 0:D], 0.0)
            nc.vector.tensor_copy(srhp_sb[:, D:D + W], srh_ps[:])
            nc.sync.dma_start(srhp_dram[:], srhp_sb[:])

            # m2[w, h', dr] = maskT[w,dr] * (SLH^T[w,h'] + SRH_pad[h', w+dr+1])
            g_sb = cpool.tile([96, 32, W4], F32)  # [(s,dr), pair, w']
            with tc.tile_pool(name="mwork", bufs=1) as mpool:
                toep_sb = mpool.tile([W, HB, D], F32)
                toep_src = bass.AP(srhp_dram, 1, [[1, W], [D + W, HB], [1, D]])
                nc.sync.dma_start(toep_sb[:], toep_src)

                m_sb = mpool.tile([W, HB, D], F32)
                slht_b = slht_sb[:].unsqueeze(2).broadcast_to((W, HB, D))
                nc.vector.tensor_add(m_sb[:], toep_sb[:], slht_b)
                m2_sb = mpool.tile([W, HB, D], F32)
                mask_b = mask_v.unsqueeze(1).broadcast_to((W, HB, D))
                nc.vector.tensor_mul(m2_sb[:], m_sb[:], mask_b)

                # W-upsample: g[(s,dr), pair, w'] = sum_w m2[w, 2p+s, dr] V[w, w']
                with tc.tile_pool(name="psG", bufs=4, space="PSUM") as psG:
                    for p in range(32):
                        g_ps = psG.tile([96, W4], F32)
                        nc.tensor.matmul(g_ps[:], m2_sb[:, 2 * p:2 * p + 2, :],
                                         v_v, start=True, stop=True)
                        nc.vector.tensor_copy(g_sb[:, p, :], g_ps[:])

            # D-upsample + exp + softmax/soft-argmin, per w' chunk of 128.
            # First D-up matmul uses p=31 so its DVE wait covers ALL g copies
            # (matmuls can carry only one semaphore wait).
            with (
                tc.tile_pool(name="epool", bufs=1) as epool,
                tc.tile_pool(name="spool", bufs=1) as spool,
                tc.tile_pool(name="psF", bufs=6, space="PSUM") as psF,
            ):
                for c in range(4):
                    p_order = ([31] + list(range(31))) if c == 0 else range(32)
                    e_sb = epool.tile([128, 32, 2, DP], F32, tag="e")
                    for p in p_order:
                        f_ps = psF.tile([128, 2, DP], F32, tag="f")
                        nc.tensor.matmul(f_ps[:], g_sb[:, p, 128 * c:128 * (c + 1)],
                                         ublk_v, start=True, stop=True)
                        nc.scalar.activation(e_sb[:, p], f_ps[:], EXP)

                    ev = e_sb[:].rearrange("p a b d -> p (a b) d")
                    z_sb = spool.tile([128, HB], F32, tag="z")
                    nc.vector.reduce_sum(z_sb[:], ev, axis=AX)
                    scr = spool.tile([128, HB, DP], F32, tag="scr")
                    dvec_b = dvec_v.unsqueeze(1).broadcast_to((128, HB, DP))
                    nc.vector.tensor_mul(scr[:], ev, dvec_b)
                    nm_sb = spool.tile([128, HB], F32, tag="nm")
                    nc.vector.reduce_sum(nm_sb[:], scr[:], axis=AX)
                    rz_sb = spool.tile([128, HB], F32, tag="rz")
                    nc.vector.reciprocal(rz_sb[:], z_sb[:])
                    oc_sb = spool.tile([128, HB], F32, tag="oc")
                    nc.vector.tensor_mul(oc_sb[:], nm_sb[:], rz_sb[:])
                    nc.sync.dma_start(
                        outt_d[:].rearrange("h (c w) -> c w h", c=4)[c], oc_sb[:])
    nc.compile()
    return nc


_NC = None


def _in_maps(left, right):
    shared = _shared_mega()
    return [{"mega": _core_mega(shared, left, right, k // 4, k % 4)}
            for k in range(8)]


def kernel(left, right):
    global _NC
    left = np.asarray(left, dtype=np.float32)
    right = np.asarray(right, dtype=np.float32)
    if _NC is None:
        _NC = build_nc()

    res = run_bass_kernel_spmd(_NC, _in_maps(left, right), core_ids=list(range(8)))
    out = np.zeros((B, H4, W4), np.float32)
    for k in range(8):
        b, j = k // 4, k % 4
        out[b, HB * j:HB * (j + 1)] = res.results[k]["outt"]
    return out
